# revision 33
# baseline (speedup 1.0000x reference)
"""Trainium2 Bass kernel for the MultiLatentAttention (dense transformer) block.

Computes, for x:(4,2048,2048), mask:(4,1,2048,2048):
    q/k/v = x @ W{q,k,v} + b  (per-head, head_dim=128, 16 heads)
    q,k <- interleaved RoPE
    attn = softmax(q k^T / sqrt(2048)) * mask
    out  = (attn @ v) @ Wo + bo

Sharding: 8 cores = 4 batches x 2 head-groups (8 heads each). Each core
computes its batch's q/k/v for its 8 heads, attention, and a partial
o-projection (row-parallel over Wo). Host sums the two partials per batch
and adds bo. No device collectives.

Design (all-f16 data path, PE-roofline focused). The PE work here is
~763us/core at the f16/f32r rate (1 cycle per output column; measured on
HW: fp8 DoubleRow only folds 2 contraction tiles per instruction at the
same per-instruction cost, and fp8 operands fail the 2e-2 gate on the
attention value path, so f16 at 1 cyc/row is the effective floor). The
kernel therefore targets >92% PE occupancy:
 - Everything except PSUM accumulators is f16: halves DMA and SBUF vs
   f32. v / oT / wo / round-0 q/k stay resident in SBUF; only q/k heads
   2..7 round-trip DRAM (f16), so no phase ever waits on a spill.
 - RoPE de-interleave is folded into the W{q,k} column permutation
   host-side (q.k is invariant under a shared head-dim permutation); the
   rotation sign is folded into the sin table.
 - Attention runs one continuous software pipeline over units
   u = (round, q-window, kb-pair, head): scores(u+1) always issue before
   the denominator/AV matmuls of u — across window and round boundaries —
   so the PE never drains while ACT (exp) and DVE (mask mult) catch up.
 - Exp runs on [128,1024] tiles (ACT instruction cost is ~flat in width);
   the mask multiply runs f16 in/out to hit the DVE 2x mode.
 - The softmax denominator is a ones-stationary matmul (the only
   partition-dim reduction the hardware does at full rate), applied after
   attn@v with a fast reciprocal + DVE multiply writing oT f16 directly.
 - PSUM (8 banks) is the scarce resource: scores [P,2,512]x2, av x2,
   dn x2 exactly fill it; pool open-order places the scores pool on banks
   freed early so attention starts without waiting on the v-phase drain.
 - DMA issue rate (~0.6us/descriptor on the sync sequencer) is the
   binding constraint early on, so xt loads are one DMA per contraction
   block, with the first weight/xt chunks leading the stream.
"""

import numpy as np

B, S, H, NH = 4, 2048, 2048, 16
D = 128            # head dim
G = 2              # head groups (tensor-parallel)
HL = NH // G       # heads per core = 8
P = 128
KO = H // P        # 16 contraction blocks
SB = S // P        # 16 sequence blocks
NQ = S // 512      # 4 query-column windows
KBP = SB // 2      # 8 key-block pairs
ROPE_BASE = 10000.0
SCALE = 1.0 / np.sqrt(np.float32(H))

_CACHE = {}


def _build_program(with_bv):
    import concourse.mybir as mybir
    import concourse.tile as tile
    from concourse import bacc

    f32 = mybir.dt.float32
    f16 = mybir.dt.float16
    AF = mybir.ActivationFunctionType
    MUL = mybir.AluOpType.mult
    ADD = mybir.AluOpType.add

    nc = bacc.Bacc("TRN2", num_devices=8, debug=False, num_swdge_queues=4)

    xT = nc.dram_tensor("xT", [H, S], f16, kind="ExternalInput")
    maskT = nc.dram_tensor("maskT", [S, S], f16, kind="ExternalInput")
    wq = nc.dram_tensor("wq", [HL, P, KO * D], f16, kind="ExternalInput")
    wk = nc.dram_tensor("wk", [HL, P, KO * D], f16, kind="ExternalInput")
    wv = nc.dram_tensor("wv", [KO, P, HL * D], f16, kind="ExternalInput")
    wo = nc.dram_tensor("wo", [HL, D, H], f16, kind="ExternalInput")
    cosP = nc.dram_tensor("cosP", [P, S], f16, kind="ExternalInput")
    sinP = nc.dram_tensor("sinP", [P, S], f16, kind="ExternalInput")
    bq = nc.dram_tensor("bq", [P, HL], f32, kind="ExternalInput")
    bk = nc.dram_tensor("bk", [P, HL], f32, kind="ExternalInput")
    bv = nc.dram_tensor("bv", [P, HL * D], f16, kind="ExternalInput")
    ones_d = nc.dram_tensor("ones", [P, P], f16, kind="ExternalInput")

    qT_d = nc.dram_tensor("qT_d", [HL, P, S], f16)
    kT_d = nc.dram_tensor("kT_d", [HL, P, S], f16)

    out = nc.dram_tensor("out", [S, H], f32, kind="ExternalOutput")

    xT_r = xT.rearrange("(ko p) s -> ko p s", p=P)
    maskT_r = maskT.rearrange("(ko p) s -> ko p s", p=P)
    out_r = out.rearrange("(mo p) n -> mo p n", p=P)

    with tile.TileContext(nc) as tc:
        with tc.tile_pool(name="vres_pool", bufs=1) as vres_pool:
            v_sb = vres_pool.tile([P, SB, HL * D], f16, name="v_sb")
            # round-0 q/k stay resident in SBUF (no spill/reload), so
            # attention starts without waiting on DRAM or an xt-space WAR
            qk0 = {
                (tag, j): vres_pool.tile([P, S], f16, name=f"{tag}t0{j}")
                for tag in ("q", "k") for j in range(2)
            }

            # ---------------- phase 1: q/k projections ----------------
            with (
                tc.tile_pool(name="xt_pool", bufs=1) as xt_pool,
                tc.tile_pool(name="cs_pool", bufs=1) as cs_pool,
                tc.tile_pool(name="wv_pool", bufs=1) as wv_pool,
            ):
              # wv prefetched up front: its SBUF lives below the drain pools,
              # so the v phase never waits on the q/k drain tail
              wv_sb = wv_pool.tile([P, KO, HL * D], f16, name="wv_sb")
              xt = xt_pool.tile([P, KO, S], f16, name="xt")
              with (
                tc.tile_pool(name="w_pool", bufs=4) as w_pool,
                tc.tile_pool(name="qps_pool", bufs=2, space="PSUM") as qps_pool,
                tc.tile_pool(name="rp2_pool", bufs=2) as rp2_pool,
                tc.tile_pool(name="rp1_pool", bufs=1) as rp1_pool,
              ):
                # first weight chunk + first xt chunk lead the DMA stream so
                # the PE's first matmul starts as early as possible
                wsb0 = {}
                w0_r = {}
                for tag, w_in in (("q", wq), ("k", wk)):
                    wsb0[tag] = w_pool.tile([P, KO, D], f16, name=f"wsb0{tag}",
                                            tag="w")
                    w0_r[tag] = w_in[0].rearrange("p (ko d) -> p ko d", d=D)
                # issue order tracks the PE's consumption: head-0 q weights
                # and the first xt blocks lead; head-0 k weights (needed only
                # after the full q projection) trail the first 6 xt blocks
                nc.sync.dma_start(wsb0["q"][:, 0:4], w0_r["q"][:, 0:4])
                nc.sync.dma_start(xt[:, 0, 0:1024], xT_r[0][:, 0:1024])
                nc.sync.dma_start(xt[:, 0, 1024:2048], xT_r[0][:, 1024:2048])
                nc.sync.dma_start(xt[:, 1], xT_r[1][:, :])
                for c in range(1, 4):
                    nc.sync.dma_start(wsb0["q"][:, 4 * c:4 * (c + 1)],
                                      w0_r["q"][:, 4 * c:4 * (c + 1)])
                for kb in range(2, 6):
                    nc.sync.dma_start(xt[:, kb], xT_r[kb][:, :])
                for c in range(4):
                    nc.sync.dma_start(wsb0["k"][:, 4 * c:4 * (c + 1)],
                                      w0_r["k"][:, 4 * c:4 * (c + 1)])
                for kb in range(6, KO):
                    nc.sync.dma_start(xt[:, kb], xT_r[kb][:, :])
                cos_sb = cs_pool.tile([P, S], f16, name="cos_sb")
                sin_sb = cs_pool.tile([P, S], f16, name="sin_sb")
                nc.sync.dma_start(cos_sb[:], cosP[:, :])
                nc.sync.dma_start(sin_sb[:], sinP[:, :])
                bq_sb = cs_pool.tile([P, HL], f32, name="bq_sb")
                bk_sb = cs_pool.tile([P, HL], f32, name="bk_sb")
                nc.sync.dma_start(bq_sb[:], bq[:, :])
                nc.sync.dma_start(bk_sb[:], bk[:, :])
                if with_bv:
                    bv_sb = cs_pool.tile([P, HL * D], f16, name="bv_sb")
                    nc.sync.dma_start(bv_sb[:], bv[:, :])

                def qk_drain(ps, b_in, h, tag, spill):
                    qb = rp2_pool.tile([P, S], f16, name="qb", tag="qb")
                    # two half-reads: the first PSUM bank pair frees one ACT
                    # earlier, so the v-phase's first accumulator (which
                    # reuses it) doesn't wait for the full drain
                    for hf in range(2):
                        nc.scalar.activation(
                            qb[:, hf * 1024:(hf + 1) * 1024],
                            ps[:, 2 * hf:2 * hf + 2],
                            AF.Identity, bias=b_in[:, h:h + 1],
                        )
                    qsw = rp1_pool.tile([P, S], f16, name="qsw", tag="qsw")
                    nc.vector.tensor_copy(qsw[0:64], qb[64:128])
                    nc.vector.tensor_copy(qsw[64:128], qb[0:64])
                    t1 = rp1_pool.tile([P, S], f16, name="t1", tag="t1")
                    nc.vector.tensor_tensor(t1[:], qb[:], cos_sb[:], MUL)
                    t2 = rp1_pool.tile([P, S], f16, name="t2", tag="t2")
                    nc.vector.tensor_tensor(t2[:], qsw[:], sin_sb[:], MUL)
                    if h < 2:
                        # round-0 heads land directly in resident SBUF tiles
                        nc.vector.tensor_tensor(
                            qk0[(tag, h)][:], t1[:], t2[:], ADD
                        )
                    else:
                        rp = rp2_pool.tile([P, S], f16, name="rp", tag="rp")
                        nc.vector.tensor_tensor(rp[:], t1[:], t2[:], ADD)
                        nc.gpsimd.dma_start(spill[h][:, :], rp[:])

                for h in range(HL):
                    # stream 2 wv blocks per head behind the q/k weights so
                    # wv never delays the next head's weight arrival
                    for kb in (2 * h, 2 * h + 1):
                        nc.sync.dma_start(wv_sb[:, kb], wv[kb][:, :])
                    for tag, w_in, b_sb, spill in (
                        ("q", wq, bq_sb, qT_d), ("k", wk, bk_sb, kT_d)
                    ):
                        if h == 0:
                            wsb = wsb0[tag]
                        else:
                            wsb = w_pool.tile([P, KO, D], f16, name="wsb", tag="w")
                            nc.sync.dma_start(
                                wsb[:], w_in[h].rearrange("p (ko d) -> p ko d", d=D)
                            )
                        ps = qps_pool.tile([P, NQ, 512], f32, name="qps",
                                           tag="qps")
                        for kb in range(KO):
                            for qc in range(NQ):
                                nc.tensor.matmul(
                                    ps[:, qc],
                                    lhsT=wsb[:, kb],
                                    rhs=xt[:, kb, qc * 512:(qc + 1) * 512],
                                    start=(kb == 0),
                                    stop=(kb == KO - 1),
                                )
                        qk_drain(ps, b_sb, h, tag, spill)

              # ---------------- phase 2: v projection ----------------
              with (
                tc.tile_pool(name="vps_pool", bufs=2, space="PSUM") as vps_pool,
              ):
                for sb in range(SB):
                    ps = vps_pool.tile([P, 2, 512], f32, name="vps", tag="vps")
                    for kb in range(KO):
                        for g2 in range(2):
                            nc.tensor.matmul(
                                ps[:, g2],
                                lhsT=xt[:, kb, sb * P:(sb + 1) * P],
                                rhs=wv_sb[:, kb, g2 * 512:(g2 + 1) * 512],
                                start=(kb == 0),
                                stop=(kb == KO - 1),
                            )
                    if with_bv:
                        nc.vector.tensor_tensor(
                            v_sb[:, sb], ps[:], bv_sb[:], ADD
                        )
                    else:
                        nc.scalar.activation(v_sb[:, sb], ps[:], AF.Copy)

            # ---------------- phase 3: attention ----------------
            with (
                tc.tile_pool(name="wo_pool", bufs=1) as wo_pool,
                tc.tile_pool(name="ot_pool", bufs=1) as ot_pool,
            ):
                wo_sb = wo_pool.tile([P, HL, H], f16, name="wo_sb")
                oT_sb = ot_pool.tile([P, HL, S], f16, name="oT_sb")
                with (
                    tc.tile_pool(name="ones_pool", bufs=1) as ones_pool,
                    tc.tile_pool(name="qk_pool", bufs=8) as qk_pool,
                    tc.tile_pool(name="m_pool", bufs=6) as m_pool,
                    tc.tile_pool(name="pr_pool", bufs=6) as pr_pool,
                    tc.tile_pool(name="pm_pool", bufs=4) as pm_pool,
                    tc.tile_pool(name="av_pool", bufs=2, space="PSUM") as av_pool,
                    tc.tile_pool(name="dn_pool", bufs=2, space="PSUM") as dn_pool,
                    tc.tile_pool(name="sc_pool", bufs=2, space="PSUM") as sc_pool,
                    tc.tile_pool(name="dr_pool", bufs=4) as dr_pool,
                ):
                    ones_sb = ones_pool.tile([P, P], f16, name="ones_sb")
                    nc.sync.dma_start(ones_sb[:], ones_d[:, :])

                    def round_loads(r):
                        qts, kts = [], []
                        for j in range(2):
                            h = 2 * r + j
                            qt = qk_pool.tile([P, S], f16, name=f"qt{r}{j}",
                                              tag="qk")
                            nc.sync.dma_start(qt[:], qT_d[h])
                            kt = qk_pool.tile([P, S], f16, name=f"kt{r}{j}",
                                              tag="qk")
                            nc.sync.dma_start(kt[:], kT_d[h])
                            qts.append(qt)
                            kts.append(kt)
                        return qts, kts

                    pending = ([qk0[("q", 0)], qk0[("q", 1)]],
                               [qk0[("k", 0)], qk0[("k", 1)]])

                    # one continuous software pipeline over every unit
                    # u = (r, qc, kbp, j): scores(u+1) always issue before
                    # dn/av(u), across qc and round boundaries, so the PE
                    # never drains while ACT/DVE catch up.
                    def issue_scores(ctx, kbp, j):
                        r, qc, qts, kts, ps_av, ps_dn, mts = ctx
                        sl = slice(qc * 512, (qc + 1) * 512)
                        if j == 0:
                            mt = m_pool.tile([P, 2, 512], f16,
                                             name="mt", tag="mt")
                            for i in range(2):
                                nc.sync.dma_start(
                                    mt[:, i], maskT_r[2 * kbp + i][:, sl]
                                )
                            mts[kbp] = mt
                        ps_s = sc_pool.tile([P, 2, 512], f32,
                                            name="ps_s", tag="ps_s")
                        for i in range(2):
                            kb = 2 * kbp + i
                            nc.tensor.matmul(
                                ps_s[:, i],
                                lhsT=kts[j][:, kb * P:(kb + 1) * P],
                                rhs=qts[j][:, sl],
                                start=True,
                                stop=True,
                            )
                        pr = pr_pool.tile([P, 2, 512], f16,
                                          name="pr", tag="pr")
                        nc.scalar.activation(
                            pr[:], ps_s[:], AF.Exp, scale=float(SCALE)
                        )
                        pm = pm_pool.tile([P, 2, 512], f16,
                                          name="pm", tag="pm")
                        nc.vector.tensor_tensor(pm[:], pr[:], mts[kbp][:], MUL)
                        return (ctx, kbp, j, pr, pm)

                    def issue_dn_av(du, dn_stash):
                        ctx, kbp, j, pr, pm = du
                        r, qc, qts, kts, ps_av, ps_dn, mts = ctx
                        if j == 0:
                            # defer j0's denominator so both heads' ones-
                            # stationary matmuls run back-to-back (single
                            # effective weight load for 4 matmuls)
                            dn_stash[kbp] = pr
                        else:
                            for jj, prj in ((0, dn_stash.pop(kbp)), (1, pr)):
                                for i in range(2):
                                    nc.tensor.matmul(
                                        ps_dn[jj][:],
                                        lhsT=ones_sb[:],
                                        rhs=prj[:, i],
                                        start=(kbp == 0 and i == 0),
                                        stop=(kbp == KBP - 1 and i == 1),
                                    )
                        for i in range(2):
                            kb = 2 * kbp + i
                            nc.tensor.matmul(
                                ps_av[j][:],
                                lhsT=v_sb[:, kb, j * D + 2 * r * D:
                                          (j + 1) * D + 2 * r * D],
                                rhs=pm[:, i],
                                start=(kbp == 0 and i == 0),
                                stop=(kbp == KBP - 1 and i == 1),
                            )
                        if kbp == KBP - 1 and j == 1:
                            # this qc is fully accumulated: drain both heads
                            sl = slice(qc * 512, (qc + 1) * 512)
                            for jj in range(2):
                                h = 2 * r + jj
                                rc = dr_pool.tile([P, 512], f32, name="rc",
                                                  tag="rc")
                                nc.vector.reciprocal_approx_fast(
                                    rc[:], ps_dn[jj][:]
                                )
                                nc.vector.tensor_tensor(
                                    oT_sb[:, h, sl], ps_av[jj][:], rc[:], MUL
                                )

                    deferred = None
                    dn_stash = {}
                    for r in range(HL // 2):
                        qts, kts = pending
                        for qc in range(NQ):
                            if qc == 1 and r + 1 < HL // 2:
                                pending = round_loads(r + 1)
                            if qc == 1 and r == 2:
                                for h in range(HL):
                                    nc.sync.dma_start(wo_sb[:, h], wo[h])
                            ps_av = [
                                av_pool.tile([P, 512], f32, name=f"av{j}",
                                             tag="av")
                                for j in range(2)
                            ]
                            ps_dn = [
                                dn_pool.tile([P, 512], f32, name=f"dn{j}",
                                             tag="dn")
                                for j in range(2)
                            ]
                            ctx = (r, qc, qts, kts, ps_av, ps_dn, {})
                            for kbp in range(KBP):
                                for j in range(2):
                                    du = issue_scores(ctx, kbp, j)
                                    if deferred is not None:
                                        issue_dn_av(deferred, dn_stash)
                                    deferred = du
                    issue_dn_av(deferred, dn_stash)

                # ---------------- phase 4: output projection ----------------
                with (
                    tc.tile_pool(name="ops_pool", bufs=2, space="PSUM") as ops_pool,
                    tc.tile_pool(name="ost_pool", bufs=2) as ost_pool,
                ):
                    for m in range(SB):
                        ps = ops_pool.tile([P, NQ, 512], f32, name="ops",
                                           tag="ops")
                        for h in range(HL):
                            for nc2 in range(NQ):
                                nc.tensor.matmul(
                                    ps[:, nc2],
                                    lhsT=oT_sb[:, h, m * P:(m + 1) * P],
                                    rhs=wo_sb[:, h, nc2 * 512:(nc2 + 1) * 512],
                                    start=(h == 0),
                                    stop=(h == HL - 1),
                                )
                        od = ost_pool.tile([P, H], f32, name="od", tag="od")
                        # drain halves on ACT and DVE in parallel (both idle
                        # in this phase) so the last block's tail halves
                        nc.scalar.activation(od[:, 0:1024], ps[:, 0:2],
                                             AF.Copy)
                        nc.sync.dma_start(out_r[m][:, 0:1024], od[:, 0:1024])
                        nc.vector.tensor_copy(od[:, 1024:2048], ps[:, 2:4])
                        nc.sync.dma_start(out_r[m][:, 1024:2048],
                                          od[:, 1024:2048])

    nc.compile()
    return nc


def _get_program(with_bv):
    key = ("nc", with_bv)
    if key not in _CACHE:
        _CACHE[key] = _build_program(with_bv)
    return _CACHE[key]


def _host_inputs(x, attention_mask, Wq, bq, Wk, bk, Wv, bv, Wo, bo):
    """Build the 8 per-core input maps (core = batch*2 + head_group)."""
    f16 = np.float16
    perm = np.concatenate([np.arange(0, D, 2), np.arange(1, D, 2)])

    inv = (1.0 / (ROPE_BASE ** (np.arange(0, D, 2, dtype=np.float64) / D)))
    t = np.arange(S, dtype=np.float64)
    fr = inv[:, None] * t[None, :]          # (64, S)
    cosP = np.concatenate([np.cos(fr), np.cos(fr)], 0).astype(f16)
    # sign folded in: rope = q*cos + swap(q)*sinP with sinP negative on the
    # first 64 partitions (rope[0:64] = q[0:64]c - q[64:128]s)
    sinP = np.concatenate([-np.sin(fr), np.sin(fr)], 0).astype(f16)
    ones = np.ones((P, P), f16)

    def w_heads_perm(W, g):
        # (HL, P, KO*D): head-major, partition-major, contiguous per row
        Wg = W[:, g * HL * D:(g + 1) * HL * D].reshape(H, HL, D)
        Wg = Wg[:, :, perm].transpose(1, 0, 2)          # (HL, H, D)
        Wg = Wg.reshape(HL, KO, P, D).transpose(0, 2, 1, 3)  # (HL, P, KO, D)
        return np.ascontiguousarray(Wg.reshape(HL, P, KO * D)).astype(f16)

    def b_heads_perm(b, g):
        # (P, HL): partition-major permuted per-head bias
        bg = b[g * HL * D:(g + 1) * HL * D].reshape(HL, D)
        return np.ascontiguousarray(bg[:, perm].T).astype(np.float32)

    groups = []
    for g in range(G):
        Wv_g = Wv[:, g * HL * D:(g + 1) * HL * D]  # (H, HL*D)
        wv_r = np.ascontiguousarray(
            Wv_g.reshape(KO, P, HL * D)
        ).astype(f16)
        groups.append({
            "wq": w_heads_perm(Wq, g),
            "wk": w_heads_perm(Wk, g),
            "bq": b_heads_perm(bq, g),
            "bk": b_heads_perm(bk, g),
            "wv": wv_r,
            "bv": np.ascontiguousarray(
                np.broadcast_to(bv[g * HL * D:(g + 1) * HL * D], (P, HL * D))
            ).astype(f16),
            "wo": np.ascontiguousarray(
                Wo[g * HL * D:(g + 1) * HL * D, :].reshape(HL, D, H)
            ).astype(f16),
        })

    in_maps = []
    for b in range(B):
        xT = np.ascontiguousarray(x[b].T).astype(f16)
        maskT = np.ascontiguousarray(attention_mask[b, 0].T).astype(f16)
        for g in range(G):
            m = dict(groups[g])
            m["xT"] = xT
            m["maskT"] = maskT
            m["cosP"] = cosP
            m["sinP"] = sinP
            m["ones"] = ones
            in_maps.append(m)
    return in_maps


def kernel(x, attention_mask, Wq, bq, Wk, bk, Wv, bv, Wo, bo, _trace=False,
           _tmpdir=None):
    from concourse.bass_utils import run_bass_kernel_spmd

    with_bv = bool(np.any(bv))
    nc = _get_program(with_bv)
    in_maps = _host_inputs(
        x, attention_mask, Wq, bq, Wk, bk, Wv, bv, Wo, bo
    )
    res = run_bass_kernel_spmd(
        nc, in_maps, list(range(8)), trace=_trace, tmpdir=_tmpdir
    )
    outs = [res.results[c]["out"] for c in range(8)]
    full = np.empty((B, S, H), np.float32)
    for b in range(B):
        full[b] = outs[2 * b] + outs[2 * b + 1] + bo[None, :]
    if _trace:
        _CACHE["last_exec_time_ns"] = res.exec_time_ns
        _CACHE["last_results"] = res
    return full


# revision 35
# speedup vs baseline: 1.1997x; 1.1997x over previous
"""Trainium2 Bass kernel for the MultiLatentAttention (dense transformer) block.

Computes, for x:(4,2048,2048), mask:(4,1,2048,2048):
    q/k/v = x @ W{q,k,v} + b  (per-head, head_dim=128, 16 heads)
    q,k <- interleaved RoPE
    attn = softmax(q k^T / sqrt(2048)) * mask
    out  = (attn @ v) @ Wo + bo

Sharding: 8 cores = 4 batches x 2 head-groups (8 heads each). Each core
computes its batch's q/k/v for its 8 heads, attention, and a partial
o-projection (row-parallel over Wo). Host sums the two partials per batch
and adds bo. No device collectives.

Design (all-f16 data path, PE-roofline focused). The PE work here is
~763us/core at the f16/f32r rate (1 cycle per output column; measured on
HW: fp8 DoubleRow only folds 2 contraction tiles per instruction at the
same per-instruction cost, and fp8 operands fail the 2e-2 gate on the
attention value path, so f16 at 1 cyc/row is the effective floor). The
kernel therefore targets >92% PE occupancy:
 - Everything except PSUM accumulators is f16: halves DMA and SBUF vs
   f32. v / oT / wo / round-0 q/k stay resident in SBUF; only q/k heads
   2..7 round-trip DRAM (f16), so no phase ever waits on a spill.
 - RoPE de-interleave is folded into the W{q,k} column permutation
   host-side (q.k is invariant under a shared head-dim permutation); the
   rotation sign is folded into the sin table.
 - Attention runs one continuous software pipeline over units
   u = (round, q-window, kb-pair, head): scores(u+1) always issue before
   the denominator/AV matmuls of u — across window and round boundaries —
   so the PE never drains while ACT (exp) and DVE (mask mult) catch up.
 - Exp runs on [128,1024] tiles (ACT instruction cost is ~flat in width);
   the mask multiply runs f16 in/out to hit the DVE 2x mode.
 - The softmax denominator is a ones-stationary matmul (the only
   partition-dim reduction the hardware does at full rate), applied after
   attn@v with a fast reciprocal + DVE multiply writing oT f16 directly.
 - PSUM (8 banks) is the scarce resource: scores [P,2,512]x2, av x2,
   dn x2 exactly fill it; pool open-order places the scores pool on banks
   freed early so attention starts without waiting on the v-phase drain.
 - DMA issue rate (~0.6us/descriptor on the sync sequencer) is the
   binding constraint early on, so xt loads are one DMA per contraction
   block, with the first weight/xt chunks leading the stream.
"""

import numpy as np

B, S, H, NH = 4, 2048, 2048, 16
D = 128            # head dim
G = 2              # head groups (tensor-parallel)
HL = NH // G       # heads per core = 8
P = 128
KO = H // P        # 16 contraction blocks
SB = S // P        # 16 sequence blocks
NQ = S // 512      # 4 query-column windows
KBP = SB // 2      # 8 key-block pairs
ROPE_BASE = 10000.0
SCALE = 1.0 / np.sqrt(np.float32(H))

_CACHE = {}


def _build_program(with_bv):
    import concourse.mybir as mybir
    import concourse.tile as tile
    from concourse import bacc

    f32 = mybir.dt.float32
    f16 = mybir.dt.float16
    AF = mybir.ActivationFunctionType
    MUL = mybir.AluOpType.mult
    ADD = mybir.AluOpType.add

    nc = bacc.Bacc("TRN2", num_devices=8, debug=False, num_swdge_queues=4)

    xT = nc.dram_tensor("xT", [H, S], f16, kind="ExternalInput")
    maskT = nc.dram_tensor("maskT", [S, S], f16, kind="ExternalInput")
    wq = nc.dram_tensor("wq", [HL, P, KO * D], f16, kind="ExternalInput")
    wk = nc.dram_tensor("wk", [HL, P, KO * D], f16, kind="ExternalInput")
    wv = nc.dram_tensor("wv", [KO, P, HL * D], f16, kind="ExternalInput")
    wo = nc.dram_tensor("wo", [HL, D, H], f16, kind="ExternalInput")
    cosP = nc.dram_tensor("cosP", [P, S], f16, kind="ExternalInput")
    sinP = nc.dram_tensor("sinP", [P, S], f16, kind="ExternalInput")
    bq = nc.dram_tensor("bq", [P, HL], f32, kind="ExternalInput")
    bk = nc.dram_tensor("bk", [P, HL], f32, kind="ExternalInput")
    bv = nc.dram_tensor("bv", [P, HL * D], f16, kind="ExternalInput")
    ones_d = nc.dram_tensor("ones", [P, P], f16, kind="ExternalInput")

    qT_d = nc.dram_tensor("qT_d", [HL, P, S], f16)
    kT_d = nc.dram_tensor("kT_d", [HL, P, S], f16)

    out = nc.dram_tensor("out", [S, H], f32, kind="ExternalOutput")

    xT_r = xT.rearrange("(ko p) s -> ko p s", p=P)
    maskT_r = maskT.rearrange("(ko p) s -> ko p s", p=P)
    out_r = out.rearrange("(mo p) n -> mo p n", p=P)

    with tile.TileContext(nc) as tc:
        with tc.tile_pool(name="vres_pool", bufs=1) as vres_pool:
            v_sb = vres_pool.tile([P, SB, HL * D], f16, name="v_sb")
            # round-0 q/k stay resident in SBUF (no spill/reload), so
            # attention starts without waiting on DRAM or an xt-space WAR
            qk0 = {
                (tag, j): vres_pool.tile([P, S], f16, name=f"{tag}t0{j}")
                for tag in ("q", "k") for j in range(2)
            }

            # ---------------- phase 1: q/k projections ----------------
            with (
                tc.tile_pool(name="xt_pool", bufs=1) as xt_pool,
                tc.tile_pool(name="cs_pool", bufs=1) as cs_pool,
                tc.tile_pool(name="wv_pool", bufs=1) as wv_pool,
            ):
              # wv prefetched up front: its SBUF lives below the drain pools,
              # so the v phase never waits on the q/k drain tail
              wv_sb = wv_pool.tile([P, KO, HL * D], f16, name="wv_sb")
              xt = xt_pool.tile([P, KO, S], f16, name="xt")
              with (
                tc.tile_pool(name="w_pool", bufs=4) as w_pool,
                tc.tile_pool(name="qps_pool", bufs=2, space="PSUM") as qps_pool,
                tc.tile_pool(name="rp2_pool", bufs=2) as rp2_pool,
                tc.tile_pool(name="rp1_pool", bufs=1) as rp1_pool,
              ):
                # first weight chunk + first xt chunk lead the DMA stream so
                # the PE's first matmul starts as early as possible
                wsb0 = {}
                w0_r = {}
                for tag, w_in in (("q", wq), ("k", wk)):
                    wsb0[tag] = w_pool.tile([P, KO, D], f16, name=f"wsb0{tag}",
                                            tag="w")
                    w0_r[tag] = w_in[0].rearrange("p (ko d) -> p ko d", d=D)
                # issue order tracks the PE's consumption: head-0 q weights
                # and the first xt blocks lead; head-0 k weights (needed only
                # after the full q projection) trail the first 6 xt blocks
                nc.sync.dma_start(wsb0["q"][:, 0:4], w0_r["q"][:, 0:4])
                nc.sync.dma_start(xt[:, 0, 0:1024], xT_r[0][:, 0:1024])
                nc.sync.dma_start(xt[:, 0, 1024:2048], xT_r[0][:, 1024:2048])
                nc.sync.dma_start(xt[:, 1], xT_r[1][:, :])
                for c in range(1, 4):
                    nc.sync.dma_start(wsb0["q"][:, 4 * c:4 * (c + 1)],
                                      w0_r["q"][:, 4 * c:4 * (c + 1)])
                for kb in range(2, 6):
                    nc.sync.dma_start(xt[:, kb], xT_r[kb][:, :])
                for c in range(4):
                    nc.sync.dma_start(wsb0["k"][:, 4 * c:4 * (c + 1)],
                                      w0_r["k"][:, 4 * c:4 * (c + 1)])
                for kb in range(6, KO):
                    nc.sync.dma_start(xt[:, kb], xT_r[kb][:, :])
                cos_sb = cs_pool.tile([P, S], f16, name="cos_sb")
                sin_sb = cs_pool.tile([P, S], f16, name="sin_sb")
                nc.sync.dma_start(cos_sb[:], cosP[:, :])
                nc.sync.dma_start(sin_sb[:], sinP[:, :])
                bq_sb = cs_pool.tile([P, HL], f32, name="bq_sb")
                bk_sb = cs_pool.tile([P, HL], f32, name="bk_sb")
                nc.sync.dma_start(bq_sb[:], bq[:, :])
                nc.sync.dma_start(bk_sb[:], bk[:, :])
                if with_bv:
                    bv_sb = cs_pool.tile([P, HL * D], f16, name="bv_sb")
                    nc.sync.dma_start(bv_sb[:], bv[:, :])

                def qk_drain(ps, b_in, h, tag, spill):
                    qb = rp2_pool.tile([P, S], f16, name="qb", tag="qb")
                    nc.scalar.activation(
                        qb[:], ps[:], AF.Identity, bias=b_in[:, h:h + 1]
                    )
                    qsw = rp1_pool.tile([P, S], f16, name="qsw", tag="qsw")
                    nc.vector.tensor_copy(qsw[0:64], qb[64:128])
                    nc.vector.tensor_copy(qsw[64:128], qb[0:64])
                    t1 = rp1_pool.tile([P, S], f16, name="t1", tag="t1")
                    nc.vector.tensor_tensor(t1[:], qb[:], cos_sb[:], MUL)
                    t2 = rp1_pool.tile([P, S], f16, name="t2", tag="t2")
                    nc.vector.tensor_tensor(t2[:], qsw[:], sin_sb[:], MUL)
                    if h < 2:
                        # round-0 heads land directly in resident SBUF tiles
                        nc.vector.tensor_tensor(
                            qk0[(tag, h)][:], t1[:], t2[:], ADD
                        )
                    else:
                        rp = rp2_pool.tile([P, S], f16, name="rp", tag="rp")
                        nc.vector.tensor_tensor(rp[:], t1[:], t2[:], ADD)
                        nc.gpsimd.dma_start(spill[h][:, :], rp[:])

                for h in range(HL):
                    # stream 2 wv blocks per head behind the q/k weights so
                    # wv never delays the next head's weight arrival
                    for kb in (2 * h, 2 * h + 1):
                        nc.sync.dma_start(wv_sb[:, kb], wv[kb][:, :])
                    for tag, w_in, b_sb, spill in (
                        ("q", wq, bq_sb, qT_d), ("k", wk, bk_sb, kT_d)
                    ):
                        if h == 0:
                            wsb = wsb0[tag]
                        else:
                            wsb = w_pool.tile([P, KO, D], f16, name="wsb", tag="w")
                            nc.sync.dma_start(
                                wsb[:], w_in[h].rearrange("p (ko d) -> p ko d", d=D)
                            )
                        ps = qps_pool.tile([P, NQ, 512], f32, name="qps",
                                           tag="qps")
                        for kb in range(KO):
                            for qc in range(NQ):
                                nc.tensor.matmul(
                                    ps[:, qc],
                                    lhsT=wsb[:, kb],
                                    rhs=xt[:, kb, qc * 512:(qc + 1) * 512],
                                    start=(kb == 0),
                                    stop=(kb == KO - 1),
                                )
                        qk_drain(ps, b_sb, h, tag, spill)

              # ---------------- phase 2: v projection ----------------
              with (
                tc.tile_pool(name="vps_pool", bufs=2, space="PSUM") as vps_pool,
              ):
                for sb in range(SB):
                    ps = vps_pool.tile([P, 2, 512], f32, name="vps", tag="vps")
                    for kb in range(KO):
                        for g2 in range(2):
                            nc.tensor.matmul(
                                ps[:, g2],
                                lhsT=xt[:, kb, sb * P:(sb + 1) * P],
                                rhs=wv_sb[:, kb, g2 * 512:(g2 + 1) * 512],
                                start=(kb == 0),
                                stop=(kb == KO - 1),
                            )
                    if with_bv:
                        nc.vector.tensor_tensor(
                            v_sb[:, sb], ps[:], bv_sb[:], ADD
                        )
                    else:
                        nc.scalar.activation(v_sb[:, sb], ps[:], AF.Copy)

            # ---------------- phase 3: attention ----------------
            with (
                tc.tile_pool(name="wo_pool", bufs=1) as wo_pool,
                tc.tile_pool(name="ot_pool", bufs=1) as ot_pool,
            ):
                wo_sb = wo_pool.tile([P, HL, H], f16, name="wo_sb")
                oT_sb = ot_pool.tile([P, HL, S], f16, name="oT_sb")
                with (
                    tc.tile_pool(name="ones_pool", bufs=1) as ones_pool,
                    tc.tile_pool(name="qk_pool", bufs=8) as qk_pool,
                    tc.tile_pool(name="m_pool", bufs=6) as m_pool,
                    tc.tile_pool(name="pr_pool", bufs=6) as pr_pool,
                    tc.tile_pool(name="pm_pool", bufs=4) as pm_pool,
                    tc.tile_pool(name="av_pool", bufs=2, space="PSUM") as av_pool,
                    tc.tile_pool(name="dn_pool", bufs=2, space="PSUM") as dn_pool,
                    tc.tile_pool(name="sc_pool", bufs=2, space="PSUM") as sc_pool,
                    tc.tile_pool(name="dr_pool", bufs=4) as dr_pool,
                ):
                    ones_sb = ones_pool.tile([P, P], f16, name="ones_sb")
                    nc.sync.dma_start(ones_sb[:], ones_d[:, :])

                    def round_loads(r):
                        qts, kts = [], []
                        for j in range(2):
                            h = 2 * r + j
                            qt = qk_pool.tile([P, S], f16, name=f"qt{r}{j}",
                                              tag="qk")
                            nc.sync.dma_start(qt[:], qT_d[h])
                            kt = qk_pool.tile([P, S], f16, name=f"kt{r}{j}",
                                              tag="qk")
                            nc.sync.dma_start(kt[:], kT_d[h])
                            qts.append(qt)
                            kts.append(kt)
                        return qts, kts

                    pending = ([qk0[("q", 0)], qk0[("q", 1)]],
                               [qk0[("k", 0)], qk0[("k", 1)]])

                    # one continuous software pipeline over every unit
                    # u = (r, qc, kbp, j): scores(u+1) always issue before
                    # dn/av(u), across qc and round boundaries, so the PE
                    # never drains while ACT/DVE catch up.
                    def issue_scores(ctx, kbp, j):
                        r, qc, qts, kts, ps_av, ps_dn, mts = ctx
                        sl = slice(qc * 512, (qc + 1) * 512)
                        if j == 0:
                            mt = m_pool.tile([P, 2, 512], f16,
                                             name="mt", tag="mt")
                            for i in range(2):
                                nc.sync.dma_start(
                                    mt[:, i], maskT_r[2 * kbp + i][:, sl]
                                )
                            mts[kbp] = mt
                        ps_s = sc_pool.tile([P, 2, 512], f32,
                                            name="ps_s", tag="ps_s")
                        for i in range(2):
                            kb = 2 * kbp + i
                            nc.tensor.matmul(
                                ps_s[:, i],
                                lhsT=kts[j][:, kb * P:(kb + 1) * P],
                                rhs=qts[j][:, sl],
                                start=True,
                                stop=True,
                            )
                        pr = pr_pool.tile([P, 2, 512], f16,
                                          name="pr", tag="pr")
                        nc.scalar.activation(
                            pr[:], ps_s[:], AF.Exp, scale=float(SCALE)
                        )
                        pm = pm_pool.tile([P, 2, 512], f16,
                                          name="pm", tag="pm")
                        nc.vector.tensor_tensor(pm[:], pr[:], mts[kbp][:], MUL)
                        return (ctx, kbp, j, pr, pm)

                    def issue_dn_av(du, dn_stash):
                        ctx, kbp, j, pr, pm = du
                        r, qc, qts, kts, ps_av, ps_dn, mts = ctx
                        if j == 0:
                            # defer j0's denominator so both heads' ones-
                            # stationary matmuls run back-to-back (single
                            # effective weight load for 4 matmuls)
                            dn_stash[kbp] = pr
                        else:
                            for jj, prj in ((0, dn_stash.pop(kbp)), (1, pr)):
                                for i in range(2):
                                    nc.tensor.matmul(
                                        ps_dn[jj][:],
                                        lhsT=ones_sb[:],
                                        rhs=prj[:, i],
                                        start=(kbp == 0 and i == 0),
                                        stop=(kbp == KBP - 1 and i == 1),
                                    )
                        for i in range(2):
                            kb = 2 * kbp + i
                            nc.tensor.matmul(
                                ps_av[j][:],
                                lhsT=v_sb[:, kb, j * D + 2 * r * D:
                                          (j + 1) * D + 2 * r * D],
                                rhs=pm[:, i],
                                start=(kbp == 0 and i == 0),
                                stop=(kbp == KBP - 1 and i == 1),
                            )
                        if kbp == KBP - 1 and j == 1:
                            # this qc is fully accumulated: drain both heads
                            sl = slice(qc * 512, (qc + 1) * 512)
                            for jj in range(2):
                                h = 2 * r + jj
                                rc = dr_pool.tile([P, 512], f32, name="rc",
                                                  tag="rc")
                                nc.vector.reciprocal_approx_fast(
                                    rc[:], ps_dn[jj][:]
                                )
                                nc.vector.tensor_tensor(
                                    oT_sb[:, h, sl], ps_av[jj][:], rc[:], MUL
                                )

                    deferred = None
                    dn_stash = {}
                    for r in range(HL // 2):
                        qts, kts = pending
                        for qc in range(NQ):
                            if qc == 1 and r + 1 < HL // 2:
                                pending = round_loads(r + 1)
                            if qc == 1 and r == 2:
                                for h in range(HL):
                                    nc.sync.dma_start(wo_sb[:, h], wo[h])
                            ps_av = [
                                av_pool.tile([P, 512], f32, name=f"av{j}",
                                             tag="av")
                                for j in range(2)
                            ]
                            ps_dn = [
                                dn_pool.tile([P, 512], f32, name=f"dn{j}",
                                             tag="dn")
                                for j in range(2)
                            ]
                            ctx = (r, qc, qts, kts, ps_av, ps_dn, {})
                            for kbp in range(KBP):
                                for j in range(2):
                                    du = issue_scores(ctx, kbp, j)
                                    if deferred is not None:
                                        issue_dn_av(deferred, dn_stash)
                                    deferred = du
                    issue_dn_av(deferred, dn_stash)

                # ---------------- phase 4: output projection ----------------
                with (
                    tc.tile_pool(name="ops_pool", bufs=2, space="PSUM") as ops_pool,
                    tc.tile_pool(name="ost_pool", bufs=2) as ost_pool,
                ):
                    for m in range(SB):
                        ps = ops_pool.tile([P, NQ, 512], f32, name="ops",
                                           tag="ops")
                        for h in range(HL):
                            for nc2 in range(NQ):
                                nc.tensor.matmul(
                                    ps[:, nc2],
                                    lhsT=oT_sb[:, h, m * P:(m + 1) * P],
                                    rhs=wo_sb[:, h, nc2 * 512:(nc2 + 1) * 512],
                                    start=(h == 0),
                                    stop=(h == HL - 1),
                                )
                        od = ost_pool.tile([P, H], f32, name="od", tag="od")
                        chunks = 4 if m == SB - 1 else 2
                        cw = H // chunks
                        for hf in range(chunks):
                            sl = slice(hf * cw, (hf + 1) * cw)
                            nc.scalar.activation(
                                od[:, sl],
                                ps[:, hf * (NQ // chunks):
                                   (hf + 1) * (NQ // chunks)],
                                AF.Copy,
                            )
                            nc.sync.dma_start(out_r[m][:, sl], od[:, sl])

    nc.compile()
    return nc


def _get_program(with_bv):
    key = ("nc", with_bv)
    if key not in _CACHE:
        _CACHE[key] = _build_program(with_bv)
    return _CACHE[key]


def _host_inputs(x, attention_mask, Wq, bq, Wk, bk, Wv, bv, Wo, bo):
    """Build the 8 per-core input maps (core = batch*2 + head_group)."""
    f16 = np.float16
    perm = np.concatenate([np.arange(0, D, 2), np.arange(1, D, 2)])

    inv = (1.0 / (ROPE_BASE ** (np.arange(0, D, 2, dtype=np.float64) / D)))
    t = np.arange(S, dtype=np.float64)
    fr = inv[:, None] * t[None, :]          # (64, S)
    cosP = np.concatenate([np.cos(fr), np.cos(fr)], 0).astype(f16)
    # sign folded in: rope = q*cos + swap(q)*sinP with sinP negative on the
    # first 64 partitions (rope[0:64] = q[0:64]c - q[64:128]s)
    sinP = np.concatenate([-np.sin(fr), np.sin(fr)], 0).astype(f16)
    ones = np.ones((P, P), f16)

    def w_heads_perm(W, g):
        # (HL, P, KO*D): head-major, partition-major, contiguous per row
        Wg = W[:, g * HL * D:(g + 1) * HL * D].reshape(H, HL, D)
        Wg = Wg[:, :, perm].transpose(1, 0, 2)          # (HL, H, D)
        Wg = Wg.reshape(HL, KO, P, D).transpose(0, 2, 1, 3)  # (HL, P, KO, D)
        return np.ascontiguousarray(Wg.reshape(HL, P, KO * D)).astype(f16)

    def b_heads_perm(b, g):
        # (P, HL): partition-major permuted per-head bias
        bg = b[g * HL * D:(g + 1) * HL * D].reshape(HL, D)
        return np.ascontiguousarray(bg[:, perm].T).astype(np.float32)

    groups = []
    for g in range(G):
        Wv_g = Wv[:, g * HL * D:(g + 1) * HL * D]  # (H, HL*D)
        wv_r = np.ascontiguousarray(
            Wv_g.reshape(KO, P, HL * D)
        ).astype(f16)
        groups.append({
            "wq": w_heads_perm(Wq, g),
            "wk": w_heads_perm(Wk, g),
            "bq": b_heads_perm(bq, g),
            "bk": b_heads_perm(bk, g),
            "wv": wv_r,
            "bv": np.ascontiguousarray(
                np.broadcast_to(bv[g * HL * D:(g + 1) * HL * D], (P, HL * D))
            ).astype(f16),
            "wo": np.ascontiguousarray(
                Wo[g * HL * D:(g + 1) * HL * D, :].reshape(HL, D, H)
            ).astype(f16),
        })

    in_maps = []
    for b in range(B):
        xT = np.ascontiguousarray(x[b].T).astype(f16)
        maskT = np.ascontiguousarray(attention_mask[b, 0].T).astype(f16)
        for g in range(G):
            m = dict(groups[g])
            m["xT"] = xT
            m["maskT"] = maskT
            m["cosP"] = cosP
            m["sinP"] = sinP
            m["ones"] = ones
            in_maps.append(m)
    return in_maps


def kernel(x, attention_mask, Wq, bq, Wk, bk, Wv, bv, Wo, bo, _trace=False,
           _tmpdir=None):
    from concourse.bass_utils import run_bass_kernel_spmd

    with_bv = bool(np.any(bv))
    nc = _get_program(with_bv)
    in_maps = _host_inputs(
        x, attention_mask, Wq, bq, Wk, bk, Wv, bv, Wo, bo
    )
    res = run_bass_kernel_spmd(
        nc, in_maps, list(range(8)), trace=_trace, tmpdir=_tmpdir
    )
    outs = [res.results[c]["out"] for c in range(8)]
    full = np.empty((B, S, H), np.float32)
    for b in range(B):
        full[b] = outs[2 * b] + outs[2 * b + 1] + bo[None, :]
    if _trace:
        _CACHE["last_exec_time_ns"] = res.exec_time_ns
        _CACHE["last_results"] = res
    return full


# revision 37
# speedup vs baseline: 1.2030x; 1.0028x over previous
"""Trainium2 Bass kernel for the MultiLatentAttention (dense transformer) block.

Computes, for x:(4,2048,2048), mask:(4,1,2048,2048):
    q/k/v = x @ W{q,k,v} + b  (per-head, head_dim=128, 16 heads)
    q,k <- interleaved RoPE
    attn = softmax(q k^T / sqrt(2048)) * mask
    out  = (attn @ v) @ Wo + bo

Sharding: 8 cores = 4 batches x 2 head-groups (8 heads each). Each core
computes its batch's q/k/v for its 8 heads, attention, and a partial
o-projection (row-parallel over Wo). Host sums the two partials per batch
and adds bo. No device collectives.

Design (all-f16 data path, PE-roofline focused). The PE work here is
~763us/core at the f16/f32r rate (1 cycle per output column; measured on
HW: fp8 DoubleRow only folds 2 contraction tiles per instruction at the
same per-instruction cost, and fp8 operands fail the 2e-2 gate on the
attention value path, so f16 at 1 cyc/row is the effective floor). The
kernel therefore targets >92% PE occupancy:
 - Everything except PSUM accumulators is f16: halves DMA and SBUF vs
   f32. v / oT / wo / round-0 q/k stay resident in SBUF; only q/k heads
   2..7 round-trip DRAM (f16), so no phase ever waits on a spill.
 - RoPE de-interleave is folded into the W{q,k} column permutation
   host-side (q.k is invariant under a shared head-dim permutation); the
   rotation sign is folded into the sin table.
 - Attention runs one continuous software pipeline over units
   u = (round, q-window, kb-pair, head): scores(u+1) always issue before
   the denominator/AV matmuls of u — across window and round boundaries —
   so the PE never drains while ACT (exp) and DVE (mask mult) catch up.
 - Exp runs on [128,1024] tiles (ACT instruction cost is ~flat in width);
   the mask multiply runs f16 in/out to hit the DVE 2x mode.
 - The softmax denominator is a ones-stationary matmul (the only
   partition-dim reduction the hardware does at full rate), applied after
   attn@v with a fast reciprocal + DVE multiply writing oT f16 directly.
 - PSUM (8 banks) is the scarce resource: scores [P,2,512]x2, av x2,
   dn x2 exactly fill it; pool open-order places the scores pool on banks
   freed early so attention starts without waiting on the v-phase drain.
 - DMA issue rate (~0.6us/descriptor on the sync sequencer) is the
   binding constraint early on, so xt loads are one DMA per contraction
   block, with the first weight/xt chunks leading the stream.
"""

import numpy as np

B, S, H, NH = 4, 2048, 2048, 16
D = 128            # head dim
G = 2              # head groups (tensor-parallel)
HL = NH // G       # heads per core = 8
P = 128
KO = H // P        # 16 contraction blocks
SB = S // P        # 16 sequence blocks
NQ = S // 512      # 4 query-column windows
KBP = SB // 2      # 8 key-block pairs
ROPE_BASE = 10000.0
SCALE = 1.0 / np.sqrt(np.float32(H))

_CACHE = {}


def _build_program(with_bv):
    import concourse.mybir as mybir
    import concourse.tile as tile
    from concourse import bacc

    f32 = mybir.dt.float32
    f16 = mybir.dt.float16
    AF = mybir.ActivationFunctionType
    MUL = mybir.AluOpType.mult
    ADD = mybir.AluOpType.add

    nc = bacc.Bacc("TRN2", num_devices=8, debug=False, num_swdge_queues=4)

    xT = nc.dram_tensor("xT", [H, S], f16, kind="ExternalInput")
    maskT = nc.dram_tensor("maskT", [S, S], f16, kind="ExternalInput")
    wq = nc.dram_tensor("wq", [HL, P, KO * D], f16, kind="ExternalInput")
    wk = nc.dram_tensor("wk", [HL, P, KO * D], f16, kind="ExternalInput")
    wv = nc.dram_tensor("wv", [KO, P, HL * D], f16, kind="ExternalInput")
    wo = nc.dram_tensor("wo", [HL, D, H], f16, kind="ExternalInput")
    cosP = nc.dram_tensor("cosP", [P, S], f16, kind="ExternalInput")
    sinP = nc.dram_tensor("sinP", [P, S], f16, kind="ExternalInput")
    bq = nc.dram_tensor("bq", [P, HL], f32, kind="ExternalInput")
    bk = nc.dram_tensor("bk", [P, HL], f32, kind="ExternalInput")
    bv = nc.dram_tensor("bv", [P, HL * D], f16, kind="ExternalInput")
    ones_d = nc.dram_tensor("ones", [P, P], f16, kind="ExternalInput")

    qT_d = nc.dram_tensor("qT_d", [HL, P, S], f16)
    kT_d = nc.dram_tensor("kT_d", [HL, P, S], f16)

    out = nc.dram_tensor("out", [S, H], f32, kind="ExternalOutput")

    xT_r = xT.rearrange("(ko p) s -> ko p s", p=P)
    maskT_r = maskT.rearrange("(ko p) s -> ko p s", p=P)
    out_r = out.rearrange("(mo p) n -> mo p n", p=P)

    with tile.TileContext(nc) as tc:
        with tc.tile_pool(name="vres_pool", bufs=1) as vres_pool:
            v_sb = vres_pool.tile([P, SB, HL * D], f16, name="v_sb")
            # round-0 q/k stay resident in SBUF (no spill/reload), so
            # attention starts without waiting on DRAM or an xt-space WAR
            qk0 = {
                (tag, j): vres_pool.tile([P, S], f16, name=f"{tag}t0{j}")
                for tag in ("q", "k") for j in range(2)
            }

            # ---------------- phase 1: q/k projections ----------------
            with (
                tc.tile_pool(name="xt_pool", bufs=1) as xt_pool,
                tc.tile_pool(name="cs_pool", bufs=1) as cs_pool,
                tc.tile_pool(name="wv_pool", bufs=1) as wv_pool,
            ):
              # wv prefetched up front: its SBUF lives below the drain pools,
              # so the v phase never waits on the q/k drain tail
              wv_sb = wv_pool.tile([P, KO, HL * D], f16, name="wv_sb")
              xt = xt_pool.tile([P, KO, S], f16, name="xt")
              with (
                tc.tile_pool(name="w_pool", bufs=4) as w_pool,
                tc.tile_pool(name="qps_pool", bufs=2, space="PSUM") as qps_pool,
                tc.tile_pool(name="rp2_pool", bufs=2) as rp2_pool,
                tc.tile_pool(name="rp1_pool", bufs=1) as rp1_pool,
              ):
                # first weight chunk + first xt chunk lead the DMA stream so
                # the PE's first matmul starts as early as possible
                wsb0 = {}
                w0_r = {}
                for tag, w_in in (("q", wq), ("k", wk)):
                    wsb0[tag] = w_pool.tile([P, KO, D], f16, name=f"wsb0{tag}",
                                            tag="w")
                    w0_r[tag] = w_in[0].rearrange("p (ko d) -> p ko d", d=D)
                # issue order tracks the PE's consumption: head-0 q weights
                # and the first xt blocks lead; head-0 k weights (needed only
                # after the full q projection) trail the first 6 xt blocks
                nc.sync.dma_start(wsb0["q"][:, 0:4], w0_r["q"][:, 0:4])
                nc.sync.dma_start(xt[:, 0, 0:1024], xT_r[0][:, 0:1024])
                nc.sync.dma_start(xt[:, 0, 1024:2048], xT_r[0][:, 1024:2048])
                nc.sync.dma_start(xt[:, 1], xT_r[1][:, :])
                for c in range(1, 4):
                    nc.sync.dma_start(wsb0["q"][:, 4 * c:4 * (c + 1)],
                                      w0_r["q"][:, 4 * c:4 * (c + 1)])
                for kb in range(2, 6):
                    nc.sync.dma_start(xt[:, kb], xT_r[kb][:, :])
                for c in range(4):
                    nc.sync.dma_start(wsb0["k"][:, 4 * c:4 * (c + 1)],
                                      w0_r["k"][:, 4 * c:4 * (c + 1)])
                for kb in range(6, KO):
                    nc.sync.dma_start(xt[:, kb], xT_r[kb][:, :])
                cos_sb = cs_pool.tile([P, S], f16, name="cos_sb")
                sin_sb = cs_pool.tile([P, S], f16, name="sin_sb")
                nc.sync.dma_start(cos_sb[:], cosP[:, :])
                nc.sync.dma_start(sin_sb[:], sinP[:, :])
                bq_sb = cs_pool.tile([P, HL], f32, name="bq_sb")
                bk_sb = cs_pool.tile([P, HL], f32, name="bk_sb")
                nc.sync.dma_start(bq_sb[:], bq[:, :])
                nc.sync.dma_start(bk_sb[:], bk[:, :])
                if with_bv:
                    bv_sb = cs_pool.tile([P, HL * D], f16, name="bv_sb")
                    nc.sync.dma_start(bv_sb[:], bv[:, :])

                def qk_drain(ps, b_in, h, tag, spill):
                    qb = rp2_pool.tile([P, S], f16, name="qb", tag="qb")
                    nc.scalar.activation(
                        qb[:], ps[:], AF.Identity, bias=b_in[:, h:h + 1]
                    )
                    qsw = rp1_pool.tile([P, S], f16, name="qsw", tag="qsw")
                    nc.vector.tensor_copy(qsw[0:64], qb[64:128])
                    nc.vector.tensor_copy(qsw[64:128], qb[0:64])
                    t1 = rp1_pool.tile([P, S], f16, name="t1", tag="t1")
                    nc.vector.tensor_tensor(t1[:], qb[:], cos_sb[:], MUL)
                    t2 = rp1_pool.tile([P, S], f16, name="t2", tag="t2")
                    nc.vector.tensor_tensor(t2[:], qsw[:], sin_sb[:], MUL)
                    if h < 2:
                        # round-0 heads land directly in resident SBUF tiles
                        nc.vector.tensor_tensor(
                            qk0[(tag, h)][:], t1[:], t2[:], ADD
                        )
                    else:
                        rp = rp2_pool.tile([P, S], f16, name="rp", tag="rp")
                        nc.vector.tensor_tensor(rp[:], t1[:], t2[:], ADD)
                        nc.gpsimd.dma_start(spill[h][:, :], rp[:])

                for h in range(HL):
                    # stream 2 wv blocks per head behind the q/k weights so
                    # wv never delays the next head's weight arrival
                    for kb in (2 * h, 2 * h + 1):
                        nc.sync.dma_start(wv_sb[:, kb], wv[kb][:, :])
                    if h == HL - 1:
                        # rotation pad: shifts the qps slot parity so the
                        # banks the v-phase reuses first hold the earlier-
                        # drained h7-q accumulator instead of h7-k
                        qps_pool.tile([P, NQ, 512], f32, name="qpad",
                                      tag="qps")
                    for tag, w_in, b_sb, spill in (
                        ("q", wq, bq_sb, qT_d), ("k", wk, bk_sb, kT_d)
                    ):
                        if h == 0:
                            wsb = wsb0[tag]
                        else:
                            wsb = w_pool.tile([P, KO, D], f16, name="wsb", tag="w")
                            nc.sync.dma_start(
                                wsb[:], w_in[h].rearrange("p (ko d) -> p ko d", d=D)
                            )
                        ps = qps_pool.tile([P, NQ, 512], f32, name="qps",
                                           tag="qps")
                        for kb in range(KO):
                            for qc in range(NQ):
                                nc.tensor.matmul(
                                    ps[:, qc],
                                    lhsT=wsb[:, kb],
                                    rhs=xt[:, kb, qc * 512:(qc + 1) * 512],
                                    start=(kb == 0),
                                    stop=(kb == KO - 1),
                                )
                        qk_drain(ps, b_sb, h, tag, spill)

              # ---------------- phase 2: v projection ----------------
              with (
                tc.tile_pool(name="vps_pool", bufs=2, space="PSUM") as vps_pool,
              ):
                for sb in range(SB):
                    ps = vps_pool.tile([P, 2, 512], f32, name="vps", tag="vps")
                    for kb in range(KO):
                        for g2 in range(2):
                            nc.tensor.matmul(
                                ps[:, g2],
                                lhsT=xt[:, kb, sb * P:(sb + 1) * P],
                                rhs=wv_sb[:, kb, g2 * 512:(g2 + 1) * 512],
                                start=(kb == 0),
                                stop=(kb == KO - 1),
                            )
                    if with_bv:
                        nc.vector.tensor_tensor(
                            v_sb[:, sb], ps[:], bv_sb[:], ADD
                        )
                    else:
                        nc.scalar.activation(v_sb[:, sb], ps[:], AF.Copy)

            # ---------------- phase 3: attention ----------------
            with (
                tc.tile_pool(name="wo_pool", bufs=1) as wo_pool,
                tc.tile_pool(name="ot_pool", bufs=1) as ot_pool,
            ):
                wo_sb = wo_pool.tile([P, HL, H], f16, name="wo_sb")
                oT_sb = ot_pool.tile([P, HL, S], f16, name="oT_sb")
                with (
                    tc.tile_pool(name="ones_pool", bufs=1) as ones_pool,
                    tc.tile_pool(name="qk_pool", bufs=8) as qk_pool,
                    tc.tile_pool(name="m_pool", bufs=6) as m_pool,
                    tc.tile_pool(name="pr_pool", bufs=6) as pr_pool,
                    tc.tile_pool(name="pm_pool", bufs=4) as pm_pool,
                    tc.tile_pool(name="av_pool", bufs=2, space="PSUM") as av_pool,
                    tc.tile_pool(name="dn_pool", bufs=2, space="PSUM") as dn_pool,
                    tc.tile_pool(name="sc_pool", bufs=2, space="PSUM") as sc_pool,
                    tc.tile_pool(name="dr_pool", bufs=4) as dr_pool,
                ):
                    ones_sb = ones_pool.tile([P, P], f16, name="ones_sb")
                    nc.sync.dma_start(ones_sb[:], ones_d[:, :])

                    def round_loads(r):
                        qts, kts = [], []
                        for j in range(2):
                            h = 2 * r + j
                            qt = qk_pool.tile([P, S], f16, name=f"qt{r}{j}",
                                              tag="qk")
                            nc.sync.dma_start(qt[:], qT_d[h])
                            kt = qk_pool.tile([P, S], f16, name=f"kt{r}{j}",
                                              tag="qk")
                            nc.sync.dma_start(kt[:], kT_d[h])
                            qts.append(qt)
                            kts.append(kt)
                        return qts, kts

                    pending = ([qk0[("q", 0)], qk0[("q", 1)]],
                               [qk0[("k", 0)], qk0[("k", 1)]])

                    # one continuous software pipeline over every unit
                    # u = (r, qc, kbp, j): scores(u+1) always issue before
                    # dn/av(u), across qc and round boundaries, so the PE
                    # never drains while ACT/DVE catch up.
                    def issue_scores(ctx, kbp, j):
                        r, qc, qts, kts, ps_av, ps_dn, mts = ctx
                        sl = slice(qc * 512, (qc + 1) * 512)
                        if j == 0:
                            mt = m_pool.tile([P, 2, 512], f16,
                                             name="mt", tag="mt")
                            for i in range(2):
                                nc.sync.dma_start(
                                    mt[:, i], maskT_r[2 * kbp + i][:, sl]
                                )
                            mts[kbp] = mt
                        ps_s = sc_pool.tile([P, 2, 512], f32,
                                            name="ps_s", tag="ps_s")
                        for i in range(2):
                            kb = 2 * kbp + i
                            nc.tensor.matmul(
                                ps_s[:, i],
                                lhsT=kts[j][:, kb * P:(kb + 1) * P],
                                rhs=qts[j][:, sl],
                                start=True,
                                stop=True,
                            )
                        pr = pr_pool.tile([P, 2, 512], f16,
                                          name="pr", tag="pr")
                        nc.scalar.activation(
                            pr[:], ps_s[:], AF.Exp, scale=float(SCALE)
                        )
                        pm = pm_pool.tile([P, 2, 512], f16,
                                          name="pm", tag="pm")
                        nc.vector.tensor_tensor(pm[:], pr[:], mts[kbp][:], MUL)
                        return (ctx, kbp, j, pr, pm)

                    def issue_dn_av(du, dn_stash):
                        ctx, kbp, j, pr, pm = du
                        r, qc, qts, kts, ps_av, ps_dn, mts = ctx
                        if j == 0:
                            # defer j0's denominator so both heads' ones-
                            # stationary matmuls run back-to-back (single
                            # effective weight load for 4 matmuls)
                            dn_stash[kbp] = pr
                        else:
                            for jj, prj in ((0, dn_stash.pop(kbp)), (1, pr)):
                                for i in range(2):
                                    nc.tensor.matmul(
                                        ps_dn[jj][:],
                                        lhsT=ones_sb[:],
                                        rhs=prj[:, i],
                                        start=(kbp == 0 and i == 0),
                                        stop=(kbp == KBP - 1 and i == 1),
                                    )
                        for i in range(2):
                            kb = 2 * kbp + i
                            nc.tensor.matmul(
                                ps_av[j][:],
                                lhsT=v_sb[:, kb, j * D + 2 * r * D:
                                          (j + 1) * D + 2 * r * D],
                                rhs=pm[:, i],
                                start=(kbp == 0 and i == 0),
                                stop=(kbp == KBP - 1 and i == 1),
                            )
                        if kbp == KBP - 1 and j == 1:
                            # this qc is fully accumulated: drain both heads
                            sl = slice(qc * 512, (qc + 1) * 512)
                            for jj in range(2):
                                h = 2 * r + jj
                                rc = dr_pool.tile([P, 512], f32, name="rc",
                                                  tag="rc")
                                nc.vector.reciprocal_approx_fast(
                                    rc[:], ps_dn[jj][:]
                                )
                                nc.vector.tensor_tensor(
                                    oT_sb[:, h, sl], ps_av[jj][:], rc[:], MUL
                                )

                    deferred = None
                    dn_stash = {}
                    for r in range(HL // 2):
                        qts, kts = pending
                        for qc in range(NQ):
                            if qc == 1 and r + 1 < HL // 2:
                                pending = round_loads(r + 1)
                            if qc == 1 and r == 2:
                                for h in range(HL):
                                    nc.sync.dma_start(wo_sb[:, h], wo[h])
                            ps_av = [
                                av_pool.tile([P, 512], f32, name=f"av{j}",
                                             tag="av")
                                for j in range(2)
                            ]
                            ps_dn = [
                                dn_pool.tile([P, 512], f32, name=f"dn{j}",
                                             tag="dn")
                                for j in range(2)
                            ]
                            ctx = (r, qc, qts, kts, ps_av, ps_dn, {})
                            for kbp in range(KBP):
                                for j in range(2):
                                    du = issue_scores(ctx, kbp, j)
                                    if deferred is not None:
                                        issue_dn_av(deferred, dn_stash)
                                    deferred = du
                    issue_dn_av(deferred, dn_stash)

                # ---------------- phase 4: output projection ----------------
                with (
                    tc.tile_pool(name="ops_pool", bufs=2, space="PSUM") as ops_pool,
                    tc.tile_pool(name="ost_pool", bufs=2) as ost_pool,
                ):
                    for m in range(SB):
                        ps = ops_pool.tile([P, NQ, 512], f32, name="ops",
                                           tag="ops")
                        for h in range(HL):
                            for nc2 in range(NQ):
                                nc.tensor.matmul(
                                    ps[:, nc2],
                                    lhsT=oT_sb[:, h, m * P:(m + 1) * P],
                                    rhs=wo_sb[:, h, nc2 * 512:(nc2 + 1) * 512],
                                    start=(h == 0),
                                    stop=(h == HL - 1),
                                )
                        od = ost_pool.tile([P, H], f32, name="od", tag="od")
                        chunks = 4 if m == SB - 1 else 2
                        cw = H // chunks
                        for hf in range(chunks):
                            sl = slice(hf * cw, (hf + 1) * cw)
                            nc.scalar.activation(
                                od[:, sl],
                                ps[:, hf * (NQ // chunks):
                                   (hf + 1) * (NQ // chunks)],
                                AF.Copy,
                            )
                            nc.sync.dma_start(out_r[m][:, sl], od[:, sl])

    nc.compile()
    return nc


def _get_program(with_bv):
    key = ("nc", with_bv)
    if key not in _CACHE:
        _CACHE[key] = _build_program(with_bv)
    return _CACHE[key]


def _host_inputs(x, attention_mask, Wq, bq, Wk, bk, Wv, bv, Wo, bo):
    """Build the 8 per-core input maps (core = batch*2 + head_group)."""
    f16 = np.float16
    perm = np.concatenate([np.arange(0, D, 2), np.arange(1, D, 2)])

    inv = (1.0 / (ROPE_BASE ** (np.arange(0, D, 2, dtype=np.float64) / D)))
    t = np.arange(S, dtype=np.float64)
    fr = inv[:, None] * t[None, :]          # (64, S)
    cosP = np.concatenate([np.cos(fr), np.cos(fr)], 0).astype(f16)
    # sign folded in: rope = q*cos + swap(q)*sinP with sinP negative on the
    # first 64 partitions (rope[0:64] = q[0:64]c - q[64:128]s)
    sinP = np.concatenate([-np.sin(fr), np.sin(fr)], 0).astype(f16)
    ones = np.ones((P, P), f16)

    def w_heads_perm(W, g):
        # (HL, P, KO*D): head-major, partition-major, contiguous per row
        Wg = W[:, g * HL * D:(g + 1) * HL * D].reshape(H, HL, D)
        Wg = Wg[:, :, perm].transpose(1, 0, 2)          # (HL, H, D)
        Wg = Wg.reshape(HL, KO, P, D).transpose(0, 2, 1, 3)  # (HL, P, KO, D)
        return np.ascontiguousarray(Wg.reshape(HL, P, KO * D)).astype(f16)

    def b_heads_perm(b, g):
        # (P, HL): partition-major permuted per-head bias
        bg = b[g * HL * D:(g + 1) * HL * D].reshape(HL, D)
        return np.ascontiguousarray(bg[:, perm].T).astype(np.float32)

    groups = []
    for g in range(G):
        Wv_g = Wv[:, g * HL * D:(g + 1) * HL * D]  # (H, HL*D)
        wv_r = np.ascontiguousarray(
            Wv_g.reshape(KO, P, HL * D)
        ).astype(f16)
        groups.append({
            "wq": w_heads_perm(Wq, g),
            "wk": w_heads_perm(Wk, g),
            "bq": b_heads_perm(bq, g),
            "bk": b_heads_perm(bk, g),
            "wv": wv_r,
            "bv": np.ascontiguousarray(
                np.broadcast_to(bv[g * HL * D:(g + 1) * HL * D], (P, HL * D))
            ).astype(f16),
            "wo": np.ascontiguousarray(
                Wo[g * HL * D:(g + 1) * HL * D, :].reshape(HL, D, H)
            ).astype(f16),
        })

    in_maps = []
    for b in range(B):
        xT = np.ascontiguousarray(x[b].T).astype(f16)
        maskT = np.ascontiguousarray(attention_mask[b, 0].T).astype(f16)
        for g in range(G):
            m = dict(groups[g])
            m["xT"] = xT
            m["maskT"] = maskT
            m["cosP"] = cosP
            m["sinP"] = sinP
            m["ones"] = ones
            in_maps.append(m)
    return in_maps


def kernel(x, attention_mask, Wq, bq, Wk, bk, Wv, bv, Wo, bo, _trace=False,
           _tmpdir=None):
    from concourse.bass_utils import run_bass_kernel_spmd

    with_bv = bool(np.any(bv))
    nc = _get_program(with_bv)
    in_maps = _host_inputs(
        x, attention_mask, Wq, bq, Wk, bk, Wv, bv, Wo, bo
    )
    res = run_bass_kernel_spmd(
        nc, in_maps, list(range(8)), trace=_trace, tmpdir=_tmpdir
    )
    outs = [res.results[c]["out"] for c in range(8)]
    full = np.empty((B, S, H), np.float32)
    for b in range(B):
        full[b] = outs[2 * b] + outs[2 * b + 1] + bo[None, :]
    if _trace:
        _CACHE["last_exec_time_ns"] = res.exec_time_ns
        _CACHE["last_results"] = res
    return full
